# revision 13
# baseline (speedup 1.0000x reference)
"""Trainium2 Bass kernel: LADIES mini-batch ER-GCN (2-layer relational GCN).

Contract: kernel(**inputs) takes the FULL unsharded inputs (numpy, keyed as in
setup_inputs) and returns the FULL [256, 32] float32 output.

Strategy (8 NeuronCores, relation-sharded layer 1, output-row-sharded layer 2):
  - h1 = relu(A0 @ xw + b1) dominates: A0 is [1024, 131072] f32 = 512 MB.
    Core c owns relations {2c, 2c+1} = a 64 MB column block of A0.  The
    kernel is HBM-bound, so 5/8 of the contraction rows stream as fp8-e3m4
    and 3/8 as bf16 (23 MB/core; measured output error ~1.2e-2 absmax vs
    the 2e-2 gate).  Both precisions carry an exact x1024 scale (fp8-e3m4's
    dynamic range needs it; 2^10 is lossless in bf16) which is folded back
    via host-side w2 *= 2^-10 -- zero extra device work.  The block is
    host-packed so every DMA is one contiguous [128, 16*W] tile and
    transfers alternate between the two HWDGE queues.
  - Each core computes xw[r] = x @ w1[r] for its 2 relations on-device
    (PSUM->SBUF casts alternate vector/scalar engines so the chain is never
    the pacer; all stream matmuls are dependency-pinned after the casts so
    they cannot interleave into the xw pipeline and stall the PE FIFO).
  - h1.T partials accumulate over three n1-column chunks [448, 448, 128],
    AllReduced as each chunk's contraction finishes; a barrier AllReduce at
    kernel start absorbs collective first-call latency + inter-core launch
    skew, so only the last (tiny, 32 KB) AllReduce is exposed at the tail.
  - Layer 2: after the AllReduce every core has full h1; core c computes
    out.T[:, 32c:32c+32] against a host-packed A1.T column block -- no
    second collective; the host concatenates the slices.  All layer-2 PE
    work is pinned after the last stream matmul.
"""

import numpy as np
import ml_dtypes

# Problem dimensions (fixed by the problem spec).
R, NB = 16, 16
N2, N1, NOUT = 8192, 1024, 256
F, E, C = 128, 64, 32

NCORES = 8
RPC = R // NCORES            # relations per core = 2
KPC = RPC * N2               # layer-1 contraction rows per core = 16384
NKT = KPC // 128             # k-tiles per core = 128
NB2 = N2 // 128              # n2-blocks per relation = 64
NOPC = NOUT // NCORES        # output rows per core = 32

CHUNKS = [512, 384, 128]     # n1-column chunks (multiples of 128!)
G = 16                       # k-tiles per A0 DMA group
NGRP = NKT // G              # DMA groups per chunk = 8
F8G = 5                      # groups 0..F8G-1 of each chunk stream as fp8
WMAX = max(CHUNKS)
A0_SCALE = 1024.0            # exact power-of-2; folded into w2 on the host

_cache = {}
last_results = None          # BassKernelResults from the most recent run


def _build_module():
    import concourse.bacc as bacc
    import concourse.tile as tile
    import concourse.mybir as mybir
    from concourse.tile import add_dep_helper

    f32 = mybir.dt.float32
    bf16 = mybir.dt.bfloat16
    f8 = mybir.dt.float8e3

    nc = bacc.Bacc("TRN2", target_bir_lowering=False, debug=False,
                   num_devices=NCORES)

    n_f8 = sum(F8G * G * w for w in CHUNKS)
    n_bf = sum((NGRP - F8G) * G * w for w in CHUNKS)
    xt = nc.dram_tensor("xt", [F, N2], bf16, kind="ExternalInput")
    a0q = nc.dram_tensor("a0q", [128, n_f8], f8, kind="ExternalInput")
    a0h = nc.dram_tensor("a0h", [128, n_bf], bf16, kind="ExternalInput")
    a1t = nc.dram_tensor("a1t", [128, NKT * C], bf16, kind="ExternalInput")
    w1c = nc.dram_tensor("w1c", [F, RPC * E], bf16, kind="ExternalInput")
    w2a = nc.dram_tensor("w2a", [E, R * C], bf16, kind="ExternalInput")
    b1 = nc.dram_tensor("b1", [E, 1], f32, kind="ExternalInput")
    b2 = nc.dram_tensor("b2", [C, 1], f32, kind="ExternalInput")
    outT = nc.dram_tensor("outT", [C, NOPC], f32, kind="ExternalOutput")

    rg = [list(range(NCORES))]
    add_op = mybir.AluOpType.add
    max_op = mybir.AluOpType.max

    with tile.TileContext(nc) as tc:
        with (
            tc.tile_pool(name="const", bufs=1) as constp,
            tc.tile_pool(name="xtp", bufs=1) as xtp,
            tc.tile_pool(name="xwp", bufs=1) as xwp,
            tc.tile_pool(name="a0qp", bufs=6) as a0qp,
            tc.tile_pool(name="a0hp", bufs=4) as a0hp,
            tc.tile_pool(name="a1p", bufs=1) as a1p,
            tc.tile_pool(name="h1p", bufs=2) as h1p,
            tc.tile_pool(name="h1sp", bufs=1) as h1sp,
            tc.tile_pool(name="h2p", bufs=3) as h2p,
            tc.tile_pool(name="psxw", bufs=4, space="PSUM") as psxw,
            tc.tile_pool(name="psh1", bufs=2, space="PSUM") as psh1,
            tc.tile_pool(name="psh2", bufs=1, space="PSUM") as psh2,
            tc.tile_pool(name="psout", bufs=1, space="PSUM") as psoutp,
            tc.tile_pool(name="dram", bufs=1, space="DRAM") as dramp,
        ):
            # ---- pre-sync barrier: first thing on the gpsimd queue, so the
            # collective stack's first-call latency + inter-core launch skew
            # are absorbed while A0 streams.
            sync_sb = constp.tile([C, 1], f32, name="sync_sb")
            nc.vector.memset(sync_sb[:], 0.0)
            cc_s_in = dramp.tile([C, 1], f32, name="cc_s_in")
            cc_s_out = dramp.tile([C, 1], f32, name="cc_s_out",
                                  addr_space="Shared")
            nc.gpsimd.dma_start(cc_s_in[:], sync_sb[:])
            nc.gpsimd.collective_compute(
                "AllReduce", add_op, replica_groups=rg,
                ins=[cc_s_in.opt()], outs=[cc_s_out.opt()],
            )

            # ---- parameter loads ----
            # xt + w1 gate the xw matmuls that feed the whole stream: load
            # them FIRST on the fast HWDGE queues (split across both).  The
            # late-needed params ride the gpsimd queue.
            w1_sb = constp.tile([F, RPC * E], bf16, name="w1_sb")
            nc.sync.dma_start(w1_sb[:], w1c[:])
            xt_sb = xtp.tile([F, N2], bf16, name="xt_sb")
            for s in range(8):
                w = N2 // 8
                eng = nc.sync if s % 2 == 0 else nc.scalar
                eng.dma_start(xt_sb[:, s * w:(s + 1) * w],
                              xt[:, s * w:(s + 1) * w])
            b2_sb = constp.tile([C, 1], f32, name="b2_sb")
            nc.gpsimd.dma_start(b2_sb[:], b2[:])
            b1_sb = constp.tile([E, 1], f32, name="b1_sb")
            nc.gpsimd.dma_start(b1_sb[:], b1[:])
            w2_sb = constp.tile([E, R * C], bf16, name="w2_sb")
            nc.gpsimd.dma_start(w2_sb[:], w2a[:])
            a1_sb = a1p.tile([128, NKT * C], bf16, name="a1_sb")
            nc.gpsimd.dma_start(a1_sb[:], a1t[:])

            # ---- xw[kt] = x[n2-block] @ w1[r_local] (kt = rl*64+nb) ----
            # one matmul per n2-block computes BOTH relations (N=128);
            # a strided copy scatters the halves to kt=nb and kt=64+nb,
            # alternating vector/scalar engines.
            xw_sb = xwp.tile([128, NKT * E], bf16, name="xw_sb", tag="xw_sb")
            xw_v = xw_sb[:].rearrange("p (rl nb e) -> p nb rl e",
                                      rl=RPC, e=E)
            cast_tail = []       # the last cast on each engine
            for nb in range(NB2):
                ps = psxw.tile([128, RPC * E], f32, name="ps_xw", tag="ps_xw")
                nc.tensor.matmul(
                    ps[:],
                    xt_sb[:, nb * 128:(nb + 1) * 128],
                    w1_sb[:],
                    start=True, stop=True,
                )
                src = ps[:].rearrange("p (rl e) -> p rl e", e=E)
                if nb % 2 == 0:
                    cp = nc.vector.tensor_copy(xw_v[:, nb], src)
                else:
                    cp = nc.scalar.copy(xw_v[:, nb], src)
                if nb >= NB2 - 2:
                    cast_tail.append(cp)

            # collective bounce buffers (one pair per chunk)
            cc_in = [dramp.tile([E, w], f32, name=f"cc_in{ch}")
                     for ch, w in enumerate(CHUNKS)]
            cc_out = [dramp.tile([E, w], f32, name=f"cc_out{ch}",
                                 addr_space="Shared")
                      for ch, w in enumerate(CHUNKS)]

            # ---- stream phase: per chunk, contract all of A0, AllReduce ----
            last_mm = None
            gi = 0               # global group counter (queue alternation)
            off_q = 0            # column offset into packed a0q (fp8)
            off_h = 0            # column offset into packed a0h (bf16)
            for ch, w in enumerate(CHUNKS):
                ps_h1 = psh1.tile([E, 512], f32, name="ps_h1", tag="ps_h1")
                for g in range(NGRP):
                    if g < F8G:
                        a0_sb = a0qp.tile([128, G * WMAX], f8, name="a0q_sb",
                                          tag="a0q")
                        src = a0q[:, off_q:off_q + G * w]
                        off_q += G * w
                    else:
                        a0_sb = a0hp.tile([128, G * WMAX], bf16, name="a0h_sb",
                                          tag="a0h")
                        src = a0h[:, off_h:off_h + G * w]
                        off_h += G * w
                    eng = nc.sync if gi % 2 == 0 else nc.scalar
                    eng.dma_start(a0_sb[:, :G * w], src)
                    gi += 1
                    for i in range(G):
                        kt = g * G + i
                        last_mm = nc.tensor.matmul(
                            ps_h1[:, :w],
                            xw_sb[:, kt * E:(kt + 1) * E],
                            a0_sb[:, i * w:(i + 1) * w],
                            start=(kt == 0), stop=(kt == NKT - 1),
                        )
                        if kt == 0:
                            # the stream must never interleave into the xw
                            # matmul pipeline (head-of-line PE stalls)
                            for cp in cast_tail:
                                add_dep_helper(last_mm.ins, cp.ins, sync=False,
                                               reason="stream after xw casts")
                h1part = h1p.tile([E, 512], f32, name="h1part", tag="h1part")
                nc.vector.tensor_copy(h1part[:, :w], ps_h1[:, :w])
                nc.gpsimd.dma_start(cc_in[ch][:], h1part[:, :w])
                nc.gpsimd.collective_compute(
                    "AllReduce", add_op, replica_groups=rg,
                    ins=[cc_in[ch].opt()], outs=[cc_out[ch].opt()],
                )

            # ---- post phase: relu + layer 2 + output matmuls ----
            # every PE instruction here is pinned after the last stream
            # matmul so a pending AllReduce can never head-of-line-block
            # the stream pipeline.
            ps_out = psoutp.tile([C, NOPC], f32, name="ps_out", tag="ps_out")
            nfinal = R * (N1 // 128)
            ifinal = 0
            for ch, w in enumerate(CHUNKS):
                o = sum(CHUNKS[:ch])
                h1s = h1sp.tile([E, 512], f32, name=f"h1s{ch}")
                nc.sync.dma_start(h1s[:, :w], cc_out[ch][:])
                h1r = h1p.tile([E, 512], bf16, name="h1r", tag="h1r")
                nc.vector.tensor_scalar(
                    h1r[:, :w], h1s[:, :w],
                    b1_sb[:], 0.0, add_op, max_op,
                )
                for b in range(w // 128):
                    ps2 = psh2.tile([128, R * C], f32, name="ps_h2",
                                    tag="ps_h2")
                    mm = nc.tensor.matmul(
                        ps2[:],
                        h1r[:, b * 128:(b + 1) * 128],
                        w2_sb[:],
                        start=True, stop=True,
                    )
                    add_dep_helper(mm.ins, last_mm.ins, sync=False,
                                   reason="post phase strictly after stream")
                    h2t = h2p.tile([128, R * C], bf16, name="h2t", tag="h2t")
                    nc.vector.tensor_copy(h2t[:], ps2[:])
                    nb1 = o // 128 + b
                    for r in range(R):
                        t = r * (N1 // 128) + nb1
                        nc.tensor.matmul(
                            ps_out[:],
                            h2t[:, r * C:(r + 1) * C],
                            a1_sb[:, t * C:(t + 1) * C],
                            start=(ifinal == 0),
                            stop=(ifinal == nfinal - 1),
                            skip_group_check=True,
                        )
                        ifinal += 1

            # ---- bias2 + store this core's out.T slice ----
            out_sb = constp.tile([C, NOPC], f32, name="out_sb", tag="out_sb")
            nc.vector.tensor_scalar_add(out_sb[:], ps_out[:], b2_sb[:])
            nc.sync.dma_start(outT[:], out_sb[:])

    nc.compile()
    return nc


def _get_module():
    if "nc" not in _cache:
        _cache["nc"] = _build_module()
    return _cache["nc"]


def make_in_maps(X_batch, sel_idx, A0, A1, comp1, bases1, comp2, bases2,
                 bias1, bias2):
    """Host-side sharding / layout prep -> per-core input maps."""
    X_batch = np.asarray(X_batch, dtype=np.float32)
    sel_idx = np.asarray(sel_idx)
    A0 = np.asarray(A0, dtype=np.float32)
    A1 = np.asarray(A1, dtype=np.float32)
    comp1 = np.asarray(comp1, dtype=np.float32)
    bases1 = np.asarray(bases1, dtype=np.float32)
    comp2 = np.asarray(comp2, dtype=np.float32)
    bases2 = np.asarray(bases2, dtype=np.float32)
    bias1 = np.asarray(bias1, dtype=np.float32)
    bias2 = np.asarray(bias2, dtype=np.float32)

    bf16 = ml_dtypes.bfloat16
    e3m4 = ml_dtypes.float8_e3m4

    x = X_batch[sel_idx.astype(np.int64)]                    # [N2, F]
    xt_host = np.ascontiguousarray(x.T.astype(bf16))         # [F, N2]

    w1 = np.einsum("rb,bfe->rfe", comp1, bases1)             # [R, F, E]
    w2 = np.einsum("rb,bec->rec", comp2, bases2)             # [R, E, C]
    # fold the exact 1/A0_SCALE back in after layer 1
    w2a_host = np.ascontiguousarray(
        (w2.transpose(1, 0, 2).reshape(E, R * C) / A0_SCALE).astype(bf16))

    # A0 carries an exact x1024 scale: lossless in bf16, and it centers the
    # fp8-e3m4 dynamic range (sigma ~2.8, max ~15.3 < 15.5).
    a0s = A0.T * A0_SCALE                                    # [R*N2, N1] f32
    b1_host = np.ascontiguousarray(bias1.reshape(E, 1) * A0_SCALE)
    b2_host = np.ascontiguousarray(bias2.reshape(C, 1))

    a1T = np.ascontiguousarray(A1.astype(bf16).T)            # [R*N1, NOUT]

    in_maps = []
    for c in range(NCORES):
        w1c_host = np.ascontiguousarray(
            np.concatenate([w1[RPC * c + i] for i in range(RPC)],
                           axis=1).astype(bf16))
        # A0 block for this core, packed so every DMA group (ch, g) is one
        # fully-contiguous [128, G*W] tile: column j of partition p within
        # group (ch, g) maps to a0s[c*KPC + (g*G + j//W)*128 + p, o + j%W].
        # Groups 0..F8G-1 of each chunk go to the fp8 tensor, the rest bf16.
        arr = a0s[c * KPC:(c + 1) * KPC].reshape(NGRP, G, 128, N1)
        arr = arr.transpose(0, 2, 1, 3)                      # [NGRP,128,G,N1]
        cols_q, cols_h = [], []
        o = 0
        for w in CHUNKS:
            sub = arr[:, :, :, o:o + w]                      # [NGRP,128,G,w]
            for g in range(NGRP):
                blk = sub[g].reshape(128, G * w)
                if g < F8G:
                    cols_q.append(blk.astype(e3m4))
                else:
                    cols_h.append(blk.astype(bf16))
            o += w
        a0q_pack = np.ascontiguousarray(np.concatenate(cols_q, axis=1))
        a0h_pack = np.ascontiguousarray(np.concatenate(cols_h, axis=1))
        # core c's 32 output rows: pack A1.T[:, 32c:32c+32] so each k-tile is
        # a [128, 32] slice living at a1t[:, t*32:(t+1)*32]
        a1_blk = a1T[:, NOPC * c:NOPC * (c + 1)]             # [R*N1, 32]
        a1_pack = np.ascontiguousarray(
            a1_blk.reshape(NKT, 128, C).transpose(1, 0, 2).reshape(128,
                                                                   NKT * C))
        in_maps.append({
            "xt": xt_host,
            "a0q": a0q_pack,
            "a0h": a0h_pack,
            "a1t": a1_pack,
            "w1c": w1c_host,
            "w2a": w2a_host,
            "b1": b1_host,
            "b2": b2_host,
        })
    return in_maps


def kernel(X_batch, sel_idx, A0, A1, comp1, bases1, comp2, bases2,
           bias1, bias2):
    global last_results
    from concourse.bass_utils import run_bass_kernel_spmd

    in_maps = make_in_maps(X_batch, sel_idx, A0, A1, comp1, bases1,
                           comp2, bases2, bias1, bias2)
    nc = _get_module()
    res = run_bass_kernel_spmd(nc, in_maps, core_ids=list(range(NCORES)))
    last_results = res

    outT = np.concatenate([res.results[c]["outT"] for c in range(NCORES)],
                          axis=1)                            # [C, NOUT]
    return np.ascontiguousarray(outT.T)                      # [NOUT, C]
